# revision 17
# baseline (speedup 1.0000x reference)
"""Trainium2 Bass kernel for a single causal attention head.

  q = x @ Wq.T; k = pos_emb @ Wk.T; v = x @ Wv.T
  out = softmax(causal(q @ k.T / sqrt(E))) @ v

Sharding (8 cores): core c -> (batch b = c//2, half h = c%2). Core h owns the
interleaved 128-row blocks {2j+h} of batch b (queries AND keys) so causal work
is balanced across the pair. Each core projects Q/K/V for its own rows, the
pair AllGathers K/V, then each core runs attention for its own queries over all
keys. Activations are fed host-transposed and chunk-major ([P, EC*TB] with
per-partition-contiguous super-chunk slices) so belt DMAs are large contiguous
lines and every matmul has the contraction dim on partitions.

Schedule (v2): ONE interleaved belt. pe8 (fp8) supers, x (fp16) supers and the
wv/wq weight pieces ride a single HWDGE ring in an order that keeps the PE fed
continuously: K projections (fp8 DoubleRow - two E-chunks per matmul) track the
pe supers, V at x-stream pace, Q lagging one chunk. K closes ~70% into the
belt; its AllGather + readback (gpsimd SWDGE / scalar ring) hide under the x
tail. A warm-up matmul burst at program start trips the PE HAM clock gate
(4/8 -> 8/8) before the first real matmul, so the whole belt runs at 2.4 GHz.

Attention uses the transposed-scores layout: s^T[k, q] tiles so softmax
denominators come from a ones-vector matmul (partition reduction on the PE) and
the attn @ v matmul consumes exp tiles directly (no transposes). exp is applied
on the scalar engine straight out of PSUM with the 1/sqrt(E) scale fused.
MM3 (attn @ v) is deferred a few key-slots behind MM1 so the PE never
head-of-line blocks on the V readback. The program is rank-uniform: causal
boundary behaviour is data (per-core mask tiles), not control flow.
"""

import os
import sys
from contextlib import ExitStack
from dataclasses import dataclass

import numpy as np


def _ensure_path():
    try:
        import concourse.bass  # noqa: F401
    except ImportError:
        for p in ("/opt/trn_rl_repo", "/root/.axon_site/_ro/trn_rl_repo"):
            if os.path.isdir(p) and p not in sys.path:
                sys.path.insert(0, p)


_ensure_path()

import concourse.bass as bass  # noqa: E402
import concourse.mybir as mybir  # noqa: E402
import concourse.tile as tile  # noqa: E402
from concourse.masks import make_identity  # noqa: E402

P = 128
F8 = mybir.dt.float8e4
F16 = mybir.dt.float16
F32 = mybir.dt.float32
DR = mybir.MatmulPerfMode.DoubleRow


@dataclass(frozen=True)
class Cfg:
    B: int = 4
    T: int = 2048
    E: int = 4096
    H: int = 128
    QGB: int = 4  # 128-blocks per query group (matmul free dim = QGB*P <= 512)

    @property
    def NB(self):  # key/query 128-blocks per core
        return self.T // (2 * P)

    @property
    def TB(self):  # rows per core
        return self.NB * P

    @property
    def NQG(self):  # query groups per core
        return self.NB // self.QGB

    @property
    def QG(self):  # queries per group
        return self.QGB * P

    @property
    def EC(self):  # contraction chunks
        return self.E // P


FULL = Cfg()

# walrus CoreV3 setupSyncWait rejects instructions carrying more than
# MAX_SYNC_WAITS wait conditions; Tile's kernel-tail drain (and occasionally a
# body instruction) can exceed it. Excess waits are hoisted onto injected
# same-engine NoOp instructions placed immediately before the offender, which
# preserves semantics (the sequencer stalls at the carrier first).
MAX_SYNC_WAITS = 1


def _dedupe_ldweights(nc: bass.Bass):
    """Drop PE Ldweights whose stationary operand is identical to the weights
    already loaded (e.g. the two 512-column halves of one projection chunk).
    Any sync conditions move onto the following PE instruction."""
    import orjson

    n = 0
    for fn in nc.m.functions:
        for bb in fn.blocks:
            out = []
            last_sig = None
            pending_sync = None
            for inst in bb.instructions:
                if getattr(inst, "engine", None) != mybir.EngineType.PE:
                    out.append(inst)
                    continue
                d = orjson.loads(nc.instruction_to_json(inst))
                if d["opcode"] == "Ldweights":
                    sig = orjson.dumps(
                        [
                            d.get("ins"),
                            d.get("tile_position"),
                            d.get("tile_size"),
                            d.get("perf_mode"),
                        ]
                    )
                    if sig == last_sig:
                        si = inst.sync_info
                        if si and (si.on_wait or si.on_update):
                            pending_sync = si
                        n += 1
                        continue  # drop
                    last_sig = sig
                if pending_sync is not None:
                    si = inst.sync_info
                    if si is None:
                        inst.sync_info = pending_sync
                    else:
                        si.on_wait = list(pending_sync.on_wait) + list(si.on_wait)
                        si.on_update = list(pending_sync.on_update) + list(
                            si.on_update
                        )
                    pending_sync = None
                out.append(inst)
            assert pending_sync is None
            bb.instructions[:] = out
    return n


def _split_sync_waits(nc: bass.Bass, maxw: int = MAX_SYNC_WAITS):
    n = 0
    for fn in nc.m.functions:
        for bb in fn.blocks:
            out = []
            for inst in bb.instructions:
                si = inst.sync_info
                waits = list(si.on_wait) if si and si.on_wait else []
                if len(waits) > maxw:
                    excess, keep = waits[:-maxw], waits[-maxw:]
                    for k in range(0, len(excess), maxw):
                        carrier = mybir.InstNoOp(
                            name=f"{inst.name}-wsplit{n}",
                            engine=inst.engine,
                            ins=[],
                            outs=[],
                            sync_info=mybir.SyncInfo(
                                on_wait=excess[k : k + maxw], on_update=[]
                            ),
                        )
                        n += 1
                        out.append(carrier)
                    si.on_wait = keep
                out.append(inst)
            bb.instructions[:] = out
    return n


# Belt order: 'pe' supers (K path, fp8, DoubleRow pairs - even sizes),
# 'x' supers (V/Q path, fp16), 'wvq' eighth-pieces (4 chunks of [wv|wq]
# weights each), 'wk0'/'wkr' the Wk head/rest, 'qm' the boundary masks.
# K-path-first: the K AllGather is a PAIR RENDEZVOUS whose latency includes
# the cores' start skew (observed up to ~15us) plus the CC stream's global
# start barrier - K must close early enough that all of it hides under the
# x stream (v2 closed K at 72% of the belt and stalled attention 25us on the
# rendezvous). wvq pieces sit between pe supers (never two in a row) so the
# PE's phase-A idle gaps stay < the ~3.4us HAM re-throttle window.
SCS_PE = [2, 2, 2, 2, 4, 4, 4, 4, 4, 4]
SCS_X = [1, 1, 2, 2, 4, 4, 4, 4, 4, 2, 2, 1, 1]
BELT = [
    ("wk0", 0),
    ("pe", 0),
    ("pe", 1),
    ("pe", 2),
    ("pe", 3),
    ("pe", 4),
    ("pe", 5),
    ("pe", 6),
    ("pe", 7),
    ("pe", 8),
    ("pe", 9),
    ("wvq", 0),
    ("wvq", 1),
    ("qm", 0),
    ("wvq", 2),
    ("wvq", 3),
    ("x", 0),
    ("wvq", 4),
    ("x", 1),
    ("wvq", 5),
    ("x", 2),
    ("wvq", 6),
    ("x", 3),
    ("wvq", 7),
    ("x", 4),
    ("x", 5),
    ("x", 6),
    ("x", 7),
    ("x", 8),
    ("x", 9),
    ("x", 10),
    ("x", 11),
    ("x", 12),
]

N_WARM = 10  # warm-up matmuls (~4us cold) to trip the HAM clock gate


def build(cfg: Cfg, mock_cc: bool = False, reps: int = 1) -> bass.Bass:
    assert cfg.H == P
    TB, NB, EC, QG, QGB, NQG, H = (
        cfg.TB, cfg.NB, cfg.EC, cfg.QG, cfg.QGB, cfg.NQG, cfg.H,
    )
    KV = TB * H  # fp16 elements of one of {kT, v} local halves
    FD = 512  # projection matmul free dim (one PSUM bank of f32)
    NT = TB // FD

    assert sum(SCS_PE) == EC and all(n % 2 == 0 for n in SCS_PE)
    assert sum(SCS_X) == EC

    nc = bass.Bass("TRN2", target_bir_lowering=False, debug=False, num_devices=8)

    xT = nc.dram_tensor("xT", [P, EC * TB], F16, kind="ExternalInput").ap()
    pe8 = nc.dram_tensor("pe8", [P, EC * TB], F8, kind="ExternalInput").ap()
    wk8 = nc.dram_tensor("wk8", [P, EC * H], F8, kind="ExternalInput").ap()
    # per-chunk interleave [wv_e | wq_e]
    wvq = nc.dram_tensor("wvq", [P, EC * 2 * H], F16, kind="ExternalInput").ap()
    qmask = nc.dram_tensor("qmask", [P, 2 * P], F16, kind="ExternalInput").ap()
    outT = nc.dram_tensor("outT", [H, TB], F16, kind="ExternalOutput").ap()

    cc_in = nc.dram_tensor("cc_in", [2 * KV], F16).ap()
    cc_k_out = nc.dram_tensor("cc_k_out", [2, KV], F16).ap()
    cc_v_out = nc.dram_tensor("cc_v_out", [2, KV], F16).ap()

    scale = 1.0 / np.sqrt(float(cfg.E))

    # The whole belt rides ONE HWDGE ring (sync): HWDGE rings execute in FIFO
    # order per SDMA engine, so the program-order interleave above IS the wire
    # order. One ring fans each op across all 16 SDMA engines and saturates
    # the HBM rate by itself. The scalar ring carries only the K readbacks +
    # tail output stores; bounce/readback ride gpsimd (SWDGE).
    with tile.TileContext(nc) as tc, ExitStack() as ctx:
        if reps > 1:  # timing amplification harness (not used for grading)
            ctx.enter_context(tc.For_i(0, reps, 1))

        consts = ctx.enter_context(tc.tile_pool(name="consts", bufs=1))
        big = ctx.enter_context(tc.tile_pool(name="big", bufs=1))
        # Every belt super gets its own dedicated buffer (per-sc tag): no pool
        # back-pressure, so the single ring drains at line rate in program
        # order and Tile cannot hoist anything past anything.
        pe_pool = ctx.enter_context(tc.tile_pool(name="pe", bufs=1))
        x_pool = ctx.enter_context(tc.tile_pool(name="xp", bufs=1))
        sm = ctx.enter_context(tc.tile_pool(name="sm", bufs=2))

        pp_ctx = ExitStack()
        pp = pp_ctx.enter_context(tc.tile_pool(name="pp", bufs=6, space="PSUM"))
        tr_ctx = ExitStack()
        tr_ps_pool = tr_ctx.enter_context(
            tc.tile_pool(name="trp", bufs=2, space="PSUM")
        )

        wk_sb = consts.tile([P, EC, H], F8, tag="wk")
        qm_sb = consts.tile([P, 2 * P], F16, tag="qm")
        # one tile per wvq piece: whole-tile DMA writes + subtile matmul
        # reads (split-piece writes into one tile lose the dependency for
        # weight-path reads - see the wk note below)
        WQP = EC // 8  # wvq piece size in chunks
        wvq_tiles = [
            consts.tile([P, WQP, 2, H], F16, tag=f"wvq{k}", name=f"wvq_sb{k}")
            for k in range(8)
        ]

        # ---- constants (vector/gpsimd so the DMA-queue engines stay free) ----
        ones_col = consts.tile([P, 1], F16, tag="ones_col")
        nc.vector.memset(ones_col[:], 1.0)
        ones_row = consts.tile([1, P], F32, tag="ones_row")
        nc.vector.memset(ones_row[:], 1.0)
        ident = consts.tile([P, P], F16, tag="ident")
        make_identity(nc, ident[:])
        # preload the ACT Exp function table during the belt so the first
        # attention exp doesn't pay the cold-table load on the critical path
        warm = consts.tile([P, 1], F16, tag="warm")
        nc.scalar.activation(
            warm[:], ones_col[:], mybir.ActivationFunctionType.Exp
        )

        k_ps = [pp.tile([P, FD], F32, tag="pp", name=f"k_ps{i}") for i in range(NT)]
        v_ps = [pp.tile([P, FD], F32, tag="pp", name=f"v_ps{i}") for i in range(NT)]
        q_ps = [pp.tile([P, FD], F32, tag="pp", name=f"q_ps{i}") for i in range(NT)]

        # ---- PE warm-up: the HAM clock gate starts every kernel at 4/8
        # (1.2 GHz) and needs ~3.4us of sustained PE activity to open. Burn
        # that window on dummy matmuls BEFORE the first real data lands, so
        # the whole belt runs at 2.4 GHz. Target k_ps[0]: its first real
        # matmul is start=True, which overwrites whatever the warm-up left.
        wm_sb = consts.tile([P, FD], F16, tag="wm")
        nc.vector.memset(wm_sb[:], 0.0)
        with tc.high_priority():
            for w in range(N_WARM):
                nc.tensor.matmul(
                    k_ps[0][:], wm_sb[:, :P], wm_sb[:],
                    start=True, stop=True, skip_group_check=True,
                )

        # ---- the belt + projections, interleaved in program order ----
        pe_tiles = {}
        x_tiles = {}
        PE_OFF = [sum(SCS_PE[:i]) for i in range(len(SCS_PE))]
        X_OFF = [sum(SCS_X[:i]) for i in range(len(SCS_X))]
        x_chunk_of = [
            (sc, c) for sc in range(len(SCS_X)) for c in range(SCS_X[sc])
        ]
        # invariant: wvq piece k (chunks [k*WQP, (k+1)*WQP)) lands before any
        # x super consuming those chunks
        _wvq_done = -1
        _x_seen = 0
        for kind, idx in BELT:
            if kind == "wvq":
                _wvq_done = (idx + 1) * WQP - 1
            elif kind == "x":
                _x_seen = X_OFF[idx] + SCS_X[idx]
                assert _x_seen - 1 <= _wvq_done, (kind, idx)
        assert _wvq_done == EC - 1 and _x_seen == EC

        def wv_chunk(e):
            return wvq_tiles[e // WQP][:, e % WQP, 0, :]

        def wq_chunk(e):
            return wvq_tiles[e // WQP][:, e % WQP, 1, :]

        def k_mms(sc):
            n, off = SCS_PE[sc], PE_OFF[sc]
            for c in range(0, n, 2):
                e = off + c
                for i in range(NT):
                    nc.tensor.matmul(
                        k_ps[i][:],
                        wk_sb[:, e : e + 2, :],
                        pe_tiles[sc][:, c : c + 2, i * FD : (i + 1) * FD],
                        start=(e == 0),
                        stop=(e == EC - 2),
                        perf_mode=DR,
                    )

        def x_chunk_mms(ps, w_chunk, sc, c):
            e = X_OFF[sc] + c
            for i in range(NT):
                nc.tensor.matmul(
                    ps[i][:],
                    w_chunk(e),
                    x_tiles[sc][:, c, i * FD : (i + 1) * FD],
                    start=(e == 0),
                    stop=(e == EC - 1),
                )

        for kind, idx in BELT:
            if kind == "wk0":
                # ONE whole-tile DMA: split-piece writes into a tile whose
                # pair-slices feed DoubleRow LDWEIGHTS lose the write->read
                # dependency in Tile (sim: uninitialized wk_sb read; HW:
                # first-execution NaN when the race is lost). Whole-tile
                # write + subtile reads is the tile_matmul-proven pattern.
                nc.sync.dma_start(
                    wk_sb[:],
                    wk8[:].rearrange("p (e h) -> p e h", h=H),
                )
            elif kind == "wkr":
                pass
            elif kind == "wvq":
                lo, hi = idx * WQP, (idx + 1) * WQP
                nc.sync.dma_start(
                    wvq_tiles[idx][:],
                    wvq[:, lo * 2 * H : hi * 2 * H].rearrange(
                        "p (e v h) -> p e v h", v=2, h=H
                    ),
                )
            elif kind == "qm":
                nc.sync.dma_start(qm_sb[:], qmask)
            elif kind == "pe":
                n, off = SCS_PE[idx], PE_OFF[idx]
                pe_tiles[idx] = pe_pool.tile(
                    [P, n, TB], F8, tag=f"pe{idx}", name=f"pe_t{idx}"
                )
                nc.sync.dma_start(
                    pe_tiles[idx][:],
                    pe8[:, off * TB : (off + n) * TB].rearrange(
                        "p (c t) -> p c t", t=TB
                    ),
                )
                k_mms(idx)
                if idx == len(SCS_PE) - 1:
                    # ---- K done: bounce + AllGather + readback on the gpsimd
                    # SWDGE queue (no belt traffic to queue behind); high
                    # priority so Tile schedules the chain as soon as k_ps
                    # closes. Readbacks ride the idle SCALAR ring.
                    with tc.high_priority():
                        kT_loc = big.tile([P, TB], F16, tag="kT_loc")
                        for i in range(NT):
                            nc.vector.tensor_copy(
                                kT_loc[:, i * FD : (i + 1) * FD], k_ps[i][:]
                            )
                        cc_k_in = cc_in[:KV].rearrange("(p t) -> p t", t=TB)
                        nc.gpsimd.dma_start(cc_k_in, kT_loc[:])
                        if mock_cc:
                            nc.gpsimd.dma_start(cc_k_out[0], cc_in[:KV])
                            nc.gpsimd.dma_start(cc_k_out[1], cc_in[:KV])
                        else:
                            nc.gpsimd.collective_compute(
                                "AllGather",
                                mybir.AluOpType.bypass,
                                replica_groups=[[0, 1], [2, 3], [4, 5], [6, 7]],
                                ins=[cc_in[:KV]],
                                outs=[cc_k_out[:]],
                            )
                        kT_sb = big.tile([P, 2 * TB], F16, tag="kT")
                        for r in range(2):
                            nc.scalar.dma_start(
                                kT_sb[:, r * TB : (r + 1) * TB],
                                cc_k_out[r].rearrange("(p t) -> p t", t=TB),
                            )
            elif kind == "x":
                n, off = SCS_X[idx], X_OFF[idx]
                x_t = x_pool.tile([P, n, TB], F16, tag=f"xs{idx}", name=f"x_t{idx}")
                nc.sync.dma_start(
                    x_t[:],
                    xT[:, off * TB : (off + n) * TB].rearrange(
                        "p (c t) -> p c t", t=TB
                    ),
                )
                x_tiles[idx] = x_t
                # V at stream pace, Q lagging ONE CHUNK so the post-belt Q
                # tail is a single chunk's matmuls
                for c in range(n):
                    e = off + c
                    x_chunk_mms(v_ps, wv_chunk, idx, c)
                    if e >= 1:
                        x_chunk_mms(q_ps, wq_chunk, *x_chunk_of[e - 1])

        # Q tail (one chunk): MM1 needs qT, but its PSUM->SBUF casts yield
        # DVE priority to the V chain below (critical path into CC + MM3).
        x_chunk_mms(q_ps, wq_chunk, *x_chunk_of[EC - 1])

        # ---- belt-end DVE order: vT copies (critical path into the V
        # transposes + bounce), then the qT casts that gate MM1 round 0,
        # then the v_loc copies draining the transposes.
        vT_loc = big.tile([P, TB], F16, tag="vT_loc")
        for i in range(NT):
            nc.vector.tensor_copy(vT_loc[:, i * FD : (i + 1) * FD], v_ps[i][:])
        qT_sb = big.tile([P, TB], F16, tag="qT")
        for i in range(NT):
            nc.vector.tensor_copy(qT_sb[:, i * FD : (i + 1) * FD], q_ps[i][:])

        # ---- V transposes to natural layout: they fill the PE's idle slot
        # while the qT casts run on DVE (MM1 can't start yet anyway), and
        # they put the V bounce on the wire at belt end + ~3us.
        v_loc = big.tile([P, NB, H], F16, tag="v_loc")
        for c in range(NB):
            t_ps = tr_ps_pool.tile([P, P], F16, tag="tr")
            nc.tensor.transpose(
                t_ps[:], vT_loc[:, c * P : (c + 1) * P], ident[:]
            )
            nc.vector.tensor_copy(v_loc[:, c, :], t_ps[:])

        # ---- phase C: attention, pipelined against the V gather. PSUM pools
        # are a LIFO stack: the transpose pool (top) and the 6 projection
        # banks free here; the shared-kslot score pairs (sT2, 4 banks) and
        # o/d (4 banks) allocate in their place. MM2 trails TWO rounds so the
        # PE keeps a round of MM1 lead on the ACT exp stream.
        tr_ctx.close()
        pp_ctx.close()
        o_pool = ctx.enter_context(tc.tile_pool(name="op", bufs=2, space="PSUM"))
        d_pool = ctx.enter_context(tc.tile_pool(name="dp", bufs=2, space="PSUM"))
        sT2_ctx = ExitStack()
        sT2_pool = sT2_ctx.enter_context(
            tc.tile_pool(name="sT2p", bufs=2, space="PSUM")
        )

        nk = {g: QGB * (g + 1) for g in range(NQG)}
        n_pairs = 2 * sum(nk.values())
        # MM3 is fully deferred, so every exp tile stays live until the
        # attn@v pass: one flat tile (no pool churn) holding all of them
        e_all = big.tile([P, n_pairs, QG], F16, tag="e_all")
        state = {"e_idx": 0}
        mm_idx = {g: 0 for g in range(NQG)}
        mm_cnt = {g: 2 * nk[g] for g in range(NQG)}
        mm2_pend = []  # per-round list of (g, col0, eT, first, last)
        mm3_all = []  # (g, col0, eT, kslot)
        o_ps = {}
        d_ps = {}

        # normalization prologue, per query group: 1/d as exp(-ln d) on ACT
        # (the exact DVE `reciprocal` microcode costs ~8ns/elem/lane and the
        # custom-op fast variant doesn't lower in this walrus build; two ACT
        # table passes are ~0.7us each at ~1e-3 relative accuracy, noise
        # against the fp8-dominated 7e-3 error). Runs as soon as THIS group's
        # d closes, overlapping the remaining MM1/exp stream.
        bc_sb = {}

        def normalize(g, bc_pool):
            lnd = sm.tile([1, QG], F32, tag="lnd", name=f"lnd{g}")
            nc.scalar.activation(
                lnd[:], d_ps[g][:], mybir.ActivationFunctionType.Ln
            )
            rec = sm.tile([1, QG], F32, tag="rec", name=f"rec{g}")
            nc.scalar.activation(
                rec[:], lnd[:], mybir.ActivationFunctionType.Exp, scale=-1.0
            )
            # broadcast rides a score-pair buffer (same tag): no extra banks
            bc_ps = bc_pool.tile([P, 2, QG], F32, tag="sT2", name=f"bc_ps{g}")
            nc.tensor.matmul(
                bc_ps[:, 0, :], ones_row[:], rec[:], start=True, stop=True
            )
            bc_sb[g] = sm.tile([P, QG], F32, tag="bcs", name=f"bc_sb{g}")
            nc.vector.tensor_copy(bc_sb[g][:], bc_ps[:, 0, :])

        def flush_mm2(bc_pool):
            for g, col0, eT, first, last in mm2_pend.pop(0):
                nc.tensor.matmul(
                    d_ps[g][:, col0:], ones_col[:], eT[:, col0:],
                    start=first, stop=last,
                )
                if last:
                    normalize(g, bc_pool)

        def mm2_mm3_book(g, col0, eT, kslot, round2):
            first = mm_idx[g] == 0
            last = mm_idx[g] == mm_cnt[g] - 1
            round2.append((g, col0, eT, first, last))
            mm3_all.append((g, col0, eT, kslot))
            mm_idx[g] += 1

        def shared_round(r, c, flush):
            kslot = r * NB + c
            col0 = c * P  # g0's; g1 is full-width here (c < QGB)
            sT2 = sT2_pool.tile([P, 2, QG], F32, tag="sT2", name=f"sT2_{r}_{c}")
            nc.tensor.matmul(
                sT2[:, 0, col0:],
                kT_sb[:, kslot * P : (kslot + 1) * P],
                qT_sb[:, col0:QG],
                start=True, stop=True,
            )
            nc.tensor.matmul(
                sT2[:, 1, :],
                kT_sb[:, kslot * P : (kslot + 1) * P],
                qT_sb[:, QG:],
                start=True, stop=True,
            )
            i0 = state["e_idx"]
            state["e_idx"] += 2
            round2 = []
            mm2_mm3_book(0, col0, e_all[:, i0], kslot, round2)
            mm2_mm3_book(1, 0, e_all[:, i0 + 1], kslot, round2)
            mm2_pend.append(round2)
            if flush and len(mm2_pend) > 2:
                flush_mm2(sT2_pool)
            nc.scalar.activation(
                e_all[:, i0 : i0 + 2].rearrange("p a b -> p (a b)")[:, col0:],
                sT2[:].rearrange("p a b -> p (a b)")[:, col0:],
                mybir.ActivationFunctionType.Exp, scale=scale,
            )
            # only g0's diagonal lives in the shared kslots
            nc.vector.tensor_mul(
                e_all[:, i0, col0 : col0 + P],
                e_all[:, i0, col0 : col0 + P],
                qm_sb[:, r * P : (r + 1) * P],
            )

        for g in range(NQG):
            o_ps[g] = o_pool.tile([P, QG], F32, tag="o", name=f"o_ps{g}")
            d_ps[g] = d_pool.tile([1, QG], F32, tag="d", name=f"d_ps{g}")

        # ---- V bounce + gather + readback behind the K chain on the gpsimd
        # queue; proceeds underneath the MM1/exp rounds.
        cc_v_in = cc_in[KV:].rearrange("(p m) -> p m", m=NB * H)
        nc.gpsimd.dma_start(cc_v_in, v_loc[:].rearrange("p c h -> p (c h)"))
        if mock_cc:
            nc.gpsimd.dma_start(cc_v_out[0], cc_in[KV:])
            nc.gpsimd.dma_start(cc_v_out[1], cc_in[KV:])
        else:
            nc.gpsimd.collective_compute(
                "AllGather",
                mybir.AluOpType.bypass,
                replica_groups=[[0, 1], [2, 3], [4, 5], [6, 7]],
                ins=[cc_in[KV:]],
                outs=[cc_v_out[:]],
            )
        v_sb = big.tile([P, 2 * NB, H], F16, tag="v")
        for r in range(2):
            nc.gpsimd.dma_start(
                v_sb[:, r * NB : (r + 1) * NB, :],
                cc_v_out[r].rearrange("(p c h) -> p c h", c=NB, h=H),
            )

        for r, c in [(r, c) for r in range(2) for c in range(QGB)]:
            shared_round(r, c, flush=True)

        # g1-only kslots, TWO per round in one score-pair tile from the SAME
        # pool (no pool transition, one merged exp per pair). Slot b's MM1
        # writes full width so the merged exp never reads unwritten PSUM;
        # the extra columns are causally dead and never consumed.
        for r, ca in [(r, ca) for r in range(2) for ca in range(QGB, NB, 2)]:
            cb = ca + 1
            kslot_a, kslot_b = r * NB + ca, r * NB + cb
            col0a, col0b = (ca - QGB) * P, (cb - QGB) * P
            sT2 = sT2_pool.tile(
                [P, 2, QG], F32, tag="sT2", name=f"sTg1_{r}_{ca}"
            )
            nc.tensor.matmul(
                sT2[:, 0, col0a:],
                kT_sb[:, kslot_a * P : (kslot_a + 1) * P],
                qT_sb[:, QG + col0a :],
                start=True, stop=True,
            )
            nc.tensor.matmul(
                sT2[:, 1, :],
                kT_sb[:, kslot_b * P : (kslot_b + 1) * P],
                qT_sb[:, QG:],
                start=True, stop=True,
            )
            i0 = state["e_idx"]
            state["e_idx"] += 2
            round2 = []
            mm2_mm3_book(1, col0a, e_all[:, i0], kslot_a, round2)
            mm2_mm3_book(1, col0b, e_all[:, i0 + 1], kslot_b, round2)
            mm2_pend.append(round2)
            if len(mm2_pend) > 2:
                flush_mm2(sT2_pool)
            nc.scalar.activation(
                e_all[:, i0 : i0 + 2].rearrange("p a b -> p (a b)")[:, col0a:],
                sT2[:].rearrange("p a b -> p (a b)")[:, col0a:],
                mybir.ActivationFunctionType.Exp, scale=scale,
            )
            nc.vector.tensor_mul(
                e_all[:, i0, col0a : col0a + P],
                e_all[:, i0, col0a : col0a + P],
                qm_sb[:, r * P : (r + 1) * P],
            )
            nc.vector.tensor_mul(
                e_all[:, i0 + 1, col0b : col0b + P],
                e_all[:, i0 + 1, col0b : col0b + P],
                qm_sb[:, r * P : (r + 1) * P],
            )
        while mm2_pend:
            flush_mm2(sT2_pool)
        sT2_ctx.close()

        # MM3 pass: kslots shared by both groups first (their g0/g1 entries
        # stay adjacent so the dedupe pass keeps one v-Ldweights per kslot),
        # then the g1-only kslots — g0's normalize + store overlap that tail.
        block1 = [t for t in mm3_all if t[3] % NB < QGB]
        block2 = [t for t in mm3_all if t[3] % NB >= QGB]
        order = block1 + block2
        last_at = {}
        for i, t in enumerate(order):
            last_at[t[0]] = i
        seen = set()
        for i, (g, col0, eT, kslot) in enumerate(order):
            nc.tensor.matmul(
                o_ps[g][:, col0:], v_sb[:, kslot, :], eT[:, col0:],
                start=(g not in seen), stop=(last_at[g] == i),
            )
            seen.add(g)
            if last_at[g] == i:
                oT = sm.tile([P, QG], F16, tag="oT", name=f"oT{g}")
                nc.vector.tensor_mul(oT[:], o_ps[g][:], bc_sb[g][:])
                # scalar ring: empty at the tail (the belt owns sync)
                nc.scalar.dma_start(outT[:, g * QG : (g + 1) * QG], oT[:])

    return nc


def _core_rows(cfg: Cfg, h: int) -> np.ndarray:
    j = np.arange(cfg.TB)
    return ((j // P) * 2 + h) * P + (j % P)


def _chunk_major(cfg: Cfg, A: np.ndarray) -> np.ndarray:
    # A: [E, TB] -> [P, EC*TB] with [p, e*TB + t] = A[e*P + p, t]
    return np.ascontiguousarray(
        A.reshape(cfg.EC, P, cfg.TB).transpose(1, 0, 2).reshape(P, cfg.EC * cfg.TB)
    )


def _wk_layout(cfg: Cfg, Wk) -> np.ndarray:
    import ml_dtypes

    # [P, EC*H] with [p, e*H + h] = Wk[h, e*P + p]
    lay = Wk.T.reshape(cfg.EC, P, cfg.H).transpose(1, 0, 2)
    return np.ascontiguousarray(lay.reshape(P, cfg.EC * cfg.H)).astype(
        ml_dtypes.float8_e4m3
    )


def _wvq_layout(cfg: Cfg, Wv, Wq) -> np.ndarray:
    def lay(W):
        return W.T.reshape(cfg.EC, P, cfg.H).transpose(1, 0, 2)

    out = np.empty((P, cfg.EC, 2, cfg.H), np.float32)
    out[:, :, 0] = lay(Wv)
    out[:, :, 1] = lay(Wq)
    return np.ascontiguousarray(
        out.reshape(P, cfg.EC * 2 * cfg.H)
    ).astype(np.float16)


def _masks(cfg: Cfg, h: int) -> np.ndarray:
    # boundary primitives [P, 2P]: cols [:P] apply to rank-0 diagonal
    # tiles, cols [P:] to rank-1 diagonal tiles (see build() docstring)
    kt = np.arange(P)[:, None]
    qt = np.arange(P)[None, :]
    tril = (kt <= qt).astype(np.float16)
    zeros = np.zeros((P, P), np.float16)
    ones = np.ones((P, P), np.float16)
    b0 = tril if h == 0 else ones
    b1 = zeros if h == 0 else tril
    return np.concatenate([b0, b1], axis=1)


def shard_inputs(cfg: Cfg, x, pos_emb, Wq, Wk, Wv):
    import ml_dtypes

    x = np.asarray(x, dtype=np.float32)
    pos_emb = np.asarray(pos_emb, dtype=np.float32)
    wk_l = _wk_layout(cfg, np.asarray(Wk, np.float32))
    wvq_l = _wvq_layout(cfg, np.asarray(Wv, np.float32), np.asarray(Wq, np.float32))
    masks = [_masks(cfg, h) for h in range(2)]
    in_maps = []
    for core in range(8):
        b, h = core // 2, core % 2
        rows = _core_rows(cfg, h)
        in_maps.append(
            {
                "xT": _chunk_major(cfg, x[b][rows].T).astype(np.float16),
                "pe8": _chunk_major(cfg, pos_emb[b][rows].T).astype(
                    ml_dtypes.float8_e4m3
                ),
                "wk8": wk_l,
                "wvq": wvq_l,
                "qmask": masks[h],
            }
        )
    return in_maps


def unshard(cfg: Cfg, results) -> np.ndarray:
    out = np.empty((cfg.B, cfg.T, cfg.H), np.float32)
    for core in range(8):
        b, h = core // 2, core % 2
        rows = _core_rows(cfg, h)
        out[b][rows] = results[core]["outT"].T.astype(np.float32)
    return out


_NC_CACHE = {}


def _get_nc(cfg: Cfg) -> bass.Bass:
    # built once per process; _split_sync_waits is applied here (HW path only —
    # the injected carriers confuse CoreSim, which never sees walrus anyway)
    if cfg not in _NC_CACHE:
        nc = build(cfg)
        _dedupe_ldweights(nc)
        _split_sync_waits(nc)
        _NC_CACHE[cfg] = nc
    return _NC_CACHE[cfg]


def kernel(x, pos_emb, Wq, Wk, Wv) -> np.ndarray:
    from concourse.bass_utils import run_bass_kernel_spmd

    cfg = FULL
    nc = _get_nc(cfg)
    in_maps = shard_inputs(cfg, x, pos_emb, Wq, Wk, Wv)
    res = run_bass_kernel_spmd(nc, in_maps, list(range(8)))
    return unshard(cfg, res.results)
